# revision 13
# baseline (speedup 1.0000x reference)
"""Trainium2 Bass kernel for a dense transformer block (B=2,T=2048,C=1024,H=16).

Sharding: token-parallel over 8 cores with causal load balancing.
Core i handles, as queries, batch-0 rows [i*256,(i+1)*256) and batch-1 rows
[(7-i)*256,(8-i)*256). Key coverage (causal) is (i+1)*256 rows of batch 0 and
(8-i)*256 rows of batch 1 — always 2304 key rows total, so the SPMD program
has static shapes. Host reorders keys as [b0-queries(256), b1-queries(256),
b0-rest, b1-rest]; batch membership / causality is enforced with a constant
on-device causal mask for the query block plus a per-core additive key bias
(0 / -1e9) folded into the softmax exp.

All tensors are kept feature-major ("transposed", [C, tokens]) on chip so
every matmul consumes natural-layout weight tiles. Matmuls run in bf16
(fp32 accumulation in PSUM); layernorm statistics use f32r matmuls with a
ones vector (full fp32 precision at bf16 speed for N>=256).
"""

import math
from contextlib import ExitStack

import numpy as np
import ml_dtypes

import concourse.bass as bass
import concourse.mybir as mybir
import concourse.tile as tile
from concourse import bacc
from concourse.bass_utils import run_bass_kernel_spmd

F32 = mybir.dt.float32
BF16 = mybir.dt.bfloat16
F32R = mybir.dt.float32r
FP = mybir.AluOpType

B, T, C, H, D, FF = 2, 2048, 1024, 16, 64, 4096
NCORES = 8
QC = 256                 # query rows per batch per core
NQ = 2 * QC              # 512 query rows per core
NK = 9 * QC              # 2304 key rows per core (always)
P = 128
CH = C // P              # 8 feature chunks
FH = FF // P             # 32 ff chunks
NEG = -1.0e9
SM_SCALE = 1.0 / math.sqrt(D)

_NC_CACHE = {}


def _build_nc():
    nc = bacc.Bacc("TRN2", target_bir_lowering=False, debug=False)

    # ---- DRAM I/O ----
    xt = nc.dram_tensor("xt", [C, NK], F32, kind="ExternalInput")
    wq = nc.dram_tensor("wq", [CH, P, C], BF16, kind="ExternalInput")
    wk = nc.dram_tensor("wk", [CH, P, C], BF16, kind="ExternalInput")
    wv = nc.dram_tensor("wv", [CH, P, C], BF16, kind="ExternalInput")
    wo = nc.dram_tensor("wo", [CH, P, C], BF16, kind="ExternalInput")
    w1 = nc.dram_tensor("w1", [FH, P, C], BF16, kind="ExternalInput")   # [m,p,(a j)] a=8
    w2 = nc.dram_tensor("w2", [CH, P, FF], BF16, kind="ExternalInput")  # [m,p,(k j)] k=32
    bias = nc.dram_tensor("bias", [P, 8 * CH], F32, kind="ExternalInput")
    # bias columns: [bq(8) bk(8) bv(8) bo(8) b2(8) ln1g(8) ln1b(8) ln2g... ] see host packing
    b1d = nc.dram_tensor("b1", [P, FH], F32, kind="ExternalInput")
    kbias = nc.dram_tensor("kbias", [P, NK // P, 2], F32, kind="ExternalInput")
    yt = nc.dram_tensor("yt", [C, NQ], F32, kind="ExternalOutput")

    xt_r = xt[:].rearrange("(a p) n -> p a n", p=P)       # [128, 8, 2304]
    yt_r = yt[:].rearrange("(a p) n -> p a n", p=P)       # [128, 8, 512]

    NKT = NK // P          # 18 key tiles
    A_TS = 256             # LN token tile
    A_NT = NK // A_TS      # 9

    with tile.TileContext(nc) as tc, ExitStack() as E:
        consts = E.enter_context(tc.tile_pool(name="consts", bufs=1))
        # bias table [128, 64] f32: columns 0:8 bq, 8:16 bk, 16:24 bv, 24:32 bo,
        # 32:40 b2, 40:48 ln1g, 48:56 ln1b, 56:64 ln2g ; ln2b packed in b1? no:
        bias_sb = consts.tile([P, 8 * CH], F32)
        nc.sync.dma_start(out=bias_sb, in_=bias[:])
        bq_s = bias_sb[:, 0:8]
        bk_s = bias_sb[:, 8:16]
        bo_s = bias_sb[:, 24:32]
        b2_s = bias_sb[:, 32:40]
        l1g = bias_sb[:, 40:48]
        l1b = bias_sb[:, 48:56]
        l2g = bias_sb[:, 56:64]
        # ln2b stored in b1 table last col? no — pack ln2b into bias col 16:24's
        # unused... bv needs broadcast layout instead (free-dim add). Use a
        # separate row-vector const for bv; ln2b gets bias cols 16:24.
        l2b = bias_sb[:, 16:24]

        b1_sb = consts.tile([P, FH], F32)
        nc.sync.dma_start(out=b1_sb, in_=b1d[:])

        kb_sb = consts.tile([P, NKT, 2], F32)
        nc.sync.dma_start(out=kb_sb, in_=kbias[:])

        ones_sb = consts.tile([P, 1], F32)
        nc.vector.memset(ones_sb, 1.0)
        ones_bf = consts.tile([P, 1], BF16)
        nc.vector.memset(ones_bf, 1.0)
        eps_sb = consts.tile([1, 1], F32)
        nc.vector.memset(eps_sb, 1e-5)

        # bv as a broadcast matrix [128, 16*64] bf16 (free-dim add for natural-V)
        bvfull = nc.dram_tensor("bvfull", [1, C], F32, kind="ExternalInput")
        bv_row = consts.tile([1, C], F32)
        nc.sync.dma_start(out=bv_row, in_=bvfull[:])
        bv_bc = consts.tile([P, H, D], BF16)
        bv_f32 = consts.tile([P, C], F32, tag="bvtmp")
        nc.gpsimd.partition_broadcast(bv_f32, bv_row)
        nc.vector.tensor_copy(out=bv_bc.rearrange("p h d -> p (h d)"), in_=bv_f32)

        # constant causal mask for the query block: 4 tiles [128, 512] bf16
        qmask = consts.tile([P, 4, NQ], BF16)
        nc.gpsimd.memset(qmask, 0.0)
        for kt in range(4):
            half = kt // 2          # 0: b0 keys, 1: b1 keys
            sub = qmask[:, kt, half * QC:(half + 1) * QC]
            nc.gpsimd.memset(sub, 1.0)
            # keep (=1) where qpos >= kpos: f - p - (kt%2)*128 >= 0
            nc.gpsimd.affine_select(
                out=sub, in_=sub,
                compare_op=FP.is_ge, fill=0.0,
                base=-(kt % 2) * P,
                pattern=[[1, QC]],
                channel_multiplier=-1,
            )

        # ---- persistent activations ----
        big = E.enter_context(tc.tile_pool(name="big", bufs=1))
        attnT = big.tile([P, CH, NQ], BF16)      # attention branch out (pre-wo), transposed
        xmid = big.tile([P, CH, NQ], F32)        # x + attn_out (query cols)
        h2T = big.tile([P, CH, NQ], BF16)        # ln2 output

        psc = E.enter_context(tc.tile_pool(name="psc", bufs=3, space="PSUM"))

        def transposed_layernorm(src_pool_tile_fn, ntiles, ts, gaps, gapb, dst, dst_cols):
            """LN over partition dim (features) for feature-major tiles.

            src_pool_tile_fn(t) -> ([128, CH, ts] f32 AP, also-usable-scratch)
            dst: [128, CH, cols] bf16, written at dst_cols offset + t*ts
            """
            with tc.tile_pool(name="lnw", bufs=2) as lnw, \
                 tc.tile_pool(name="lnrow", bufs=3) as lnrow, \
                 tc.tile_pool(name="lnbc", bufs=3) as lnbc, \
                 tc.tile_pool(name="lnps", bufs=2, space="PSUM") as lnps:
                for t in range(ntiles):
                    x_t = src_pool_tile_fn(t)
                    ps_s = lnps.tile([1, ts], F32, tag="pss")
                    ps_s2 = lnps.tile([1, ts], F32, tag="pss2")
                    for a in range(CH):
                        sq = lnw.tile([P, ts], BF16, tag="sq")
                        nc.vector.tensor_mul(sq, x_t[:, a, :], x_t[:, a, :])
                        nc.tensor.matmul(ps_s, lhsT=ones_sb,
                                         rhs=x_t[:, a, :],
                                         start=(a == 0), stop=(a == CH - 1),
                                         skip_group_check=True)
                        nc.tensor.matmul(ps_s2, lhsT=ones_bf,
                                         rhs=sq,
                                         start=(a == 0), stop=(a == CH - 1),
                                         skip_group_check=True)
                    mu = lnrow.tile([1, ts], F32, tag="mu")
                    nc.scalar.mul(mu, ps_s, 1.0 / C)
                    ex2 = lnrow.tile([1, ts], F32, tag="ex2")
                    nc.scalar.mul(ex2, ps_s2, 1.0 / C)
                    var = lnrow.tile([1, ts], F32, tag="var")
                    nc.vector.tensor_mul(var, mu, mu)
                    nc.vector.tensor_sub(var, ex2, var)
                    sd = lnrow.tile([1, ts], F32, tag="sd")
                    nc.scalar.activation(sd, var, mybir.ActivationFunctionType.Sqrt,
                                         bias=eps_sb, scale=1.0)
                    rstd = lnrow.tile([1, ts], F32, tag="rstd")
                    nc.vector.reciprocal(rstd, sd)
                    mu_b = lnbc.tile([P, ts], F32, tag="mub")
                    nc.gpsimd.partition_broadcast(mu_b, mu)
                    rstd_b = lnbc.tile([P, ts], F32, tag="rstdb")
                    nc.gpsimd.partition_broadcast(rstd_b, rstd)
                    for a in range(CH):
                        tmp = lnw.tile([P, ts], F32, tag="lntmp")
                        nc.vector.tensor_sub(tmp, x_t[:, a, :], mu_b)
                        nc.vector.tensor_mul(tmp, tmp, rstd_b)
                        nc.vector.tensor_scalar(
                            out=dst[:, a, dst_cols + t * ts: dst_cols + t * ts + ts],
                            in0=tmp, scalar1=gaps[:, a:a + 1], scalar2=gapb[:, a:a + 1],
                            op0=FP.mult, op1=FP.add)

        # ================= stage A+B: LN1, QKV =================
        with tc.tile_pool(name="kvq", bufs=1) as kvq:
            KT_sb = kvq.tile([P, CH, NK], BF16)
            V_sb = kvq.tile([P, NKT, H, D + 1], BF16)
            QT_sb = kvq.tile([P, CH, NQ], BF16)

            with tc.tile_pool(name="hT", bufs=1) as hTp:
                hT = hTp.tile([P, CH, NK], BF16)

                # ---- A: LN1 (stream x in 9 tiles of 256 tokens) ----
                with tc.tile_pool(name="xs", bufs=2) as xs:
                    def ln1_src(t):
                        x_t = xs.tile([P, CH, A_TS], F32, tag="xt")
                        nc.sync.dma_start(
                            out=x_t, in_=xt_r[:, :, t * A_TS:(t + 1) * A_TS])
                        return x_t
                    transposed_layernorm(ln1_src, A_NT, A_TS, l1g, l1b, hT, 0)

                # ---- B: projections ----
                # K^T and Q^T (transposed outs) + V (natural out)
                with tc.tile_pool(name="wst", bufs=6) as wst, \
                     tc.tile_pool(name="wres", bufs=1) as wres:
                    # K^T: token tiles of 512 (last 256)
                    for (wdram, bsl, dst, ncols) in ((wk, bk_s, KT_sb, NK),
                                                     (wq, bq_s, QT_sb, NQ)):
                        tts = [512] * (ncols // 512) + ([256] if ncols % 512 else [])
                        t0 = 0
                        for ts_ in tts:
                            for m in range(CH):
                                wt = wst.tile([P, CH, P], BF16, tag="wt")
                                nc.sync.dma_start(out=wt, in_=wdram[m])
                                ps = psc.tile([P, 512], F32, tag="bps")
                                for k in range(CH):
                                    nc.tensor.matmul(
                                        ps[:, 0:ts_], lhsT=wt[:, k, :],
                                        rhs=hT[:, k, t0:t0 + ts_],
                                        start=(k == 0), stop=(k == CH - 1))
                                nc.vector.tensor_scalar_add(
                                    out=dst[:, m, t0:t0 + ts_], in0=ps[:, 0:ts_],
                                    scalar1=bsl[:, m:m + 1])
                            t0 += ts_

                    # V natural: wv resident [128, CH, C]
                    wv_sb = wres.tile([P, CH, C], BF16)
                    for m in range(CH):
                        nc.sync.dma_start(out=wv_sb[:, :, m * P:(m + 1) * P],
                                          in_=wv[m])
                    for kt in range(NKT):
                        for n in range(2):
                            ps = psc.tile([P, 512], F32, tag="bps")
                            for k in range(CH):
                                nc.tensor.matmul(
                                    ps, lhsT=hT[:, k, kt * P:(kt + 1) * P],
                                    rhs=wv_sb[:, k, n * 512:(n + 1) * 512],
                                    start=(k == 0), stop=(k == CH - 1))
                            nc.vector.tensor_add(
                                out=V_sb[:, kt, 8 * n:8 * n + 8, 0:D],
                                in0=ps.rearrange("p (h d) -> p h d", d=D),
                                in1=bv_bc[:, 8 * n:8 * n + 8, :])
                        nc.vector.memset(V_sb[:, kt, :, D:D + 1], 1.0)

            # ================= stage C: attention =================
            with tc.tile_pool(name="att", bufs=4) as att, \
                 tc.tile_pool(name="attbc", bufs=3) as attbc, \
                 tc.tile_pool(name="psy", bufs=2, space="PSUM") as psy, \
                 tc.tile_pool(name="pss", bufs=3, space="PSUM") as pss:
                for h in range(H):
                    mc = h // 2
                    pr0 = (h % 2) * D
                    q_h = QT_sb[pr0:pr0 + D, mc, :]
                    y_ps = psy.tile([P, NQ], F32, tag="yps")
                    for kt in range(NKT):
                        ps_s = pss.tile([P, NQ], F32, tag="sps")
                        nc.tensor.matmul(
                            ps_s, lhsT=KT_sb[pr0:pr0 + D, mc, kt * P:(kt + 1) * P],
                            rhs=q_h, start=True, stop=True)
                        p_t = att.tile([P, NQ], BF16, tag="pt")
                        for half in range(2):
                            nc.scalar.activation(
                                out=p_t[:, half * QC:(half + 1) * QC],
                                in_=ps_s[:, half * QC:(half + 1) * QC],
                                func=mybir.ActivationFunctionType.Exp,
                                bias=kb_sb[:, kt, half:half + 1], scale=SM_SCALE)
                        if kt < 4:
                            nc.vector.tensor_mul(p_t, p_t, qmask[:, kt, :])
                        nc.tensor.matmul(
                            y_ps[0:D + 1, :], lhsT=V_sb[:, kt, h, :],
                            rhs=p_t, start=(kt == 0), stop=(kt == NKT - 1))
                    rec = att.tile([1, NQ], F32, tag="rec")
                    nc.vector.reciprocal(rec, y_ps[D:D + 1, :])
                    rec_b = attbc.tile([D, NQ], F32, tag="recb")
                    nc.gpsimd.partition_broadcast(rec_b, rec)
                    nc.vector.tensor_mul(attnT[pr0:pr0 + D, mc, :],
                                         y_ps[0:D, :], rec_b)

        # ================= stage D: out-proj + residual =================
        with tc.tile_pool(name="wst2", bufs=6) as wst2, \
             tc.tile_pool(name="xqs", bufs=3) as xqs:
            for m in range(CH):
                wt = wst2.tile([P, CH, P], BF16, tag="wot")
                nc.sync.dma_start(out=wt, in_=wo[m])
                ps = psc.tile([P, NQ], F32, tag="bps")
                for k in range(CH):
                    nc.tensor.matmul(ps, lhsT=wt[:, k, :], rhs=attnT[:, k, :],
                                     start=(k == 0), stop=(k == CH - 1))
                xq_t = xqs.tile([P, NQ], F32, tag="xq")
                nc.sync.dma_start(out=xq_t, in_=xt_r[:, m, 0:NQ])
                tmp = xqs.tile([P, NQ], F32, tag="dtmp")
                nc.vector.tensor_scalar_add(out=tmp, in0=ps, scalar1=bo_s[:, m:m + 1])
                nc.vector.tensor_add(out=xmid[:, m, :], in0=tmp, in1=xq_t)

        # ================= stage E: LN2 =================
        transposed_layernorm(lambda t: xmid[:, :, t * A_TS:(t + 1) * A_TS],
                             NQ // A_TS, A_TS, l2g, l2b, h2T, 0)

        # ================= stage F: MLP up + GELU =================
        with tc.tile_pool(name="gp", bufs=1) as gp:
            g_sb = gp.tile([P, FH, NQ], BF16)
            with tc.tile_pool(name="w1s", bufs=6) as w1s:
                for m in range(FH):
                    wt = w1s.tile([P, CH, P], BF16, tag="w1t")
                    nc.sync.dma_start(out=wt, in_=w1[m])
                    ps = psc.tile([P, NQ], F32, tag="bps")
                    for k in range(CH):
                        nc.tensor.matmul(ps, lhsT=wt[:, k, :], rhs=h2T[:, k, :],
                                         start=(k == 0), stop=(k == CH - 1))
                    nc.scalar.activation(
                        out=g_sb[:, m, :], in_=ps,
                        func=mybir.ActivationFunctionType.Gelu,
                        bias=b1_sb[:, m:m + 1], scale=1.0)

            # ============= stage G: MLP down + residual =============
            with tc.tile_pool(name="w2s", bufs=3) as w2s, \
                 tc.tile_pool(name="outs", bufs=3) as outs:
                for m in range(CH):
                    wt = w2s.tile([P, FH, P], BF16, tag="w2t")
                    nc.sync.dma_start(out=wt, in_=w2[m])
                    ps = psc.tile([P, NQ], F32, tag="bps")
                    for k in range(FH):
                        nc.tensor.matmul(ps, lhsT=wt[:, k, :], rhs=g_sb[:, k, :],
                                         start=(k == 0), stop=(k == FH - 1))
                    tmp = outs.tile([P, NQ], F32, tag="otmp")
                    nc.vector.tensor_scalar_add(out=tmp, in0=ps,
                                                scalar1=b2_s[:, m:m + 1])
                    out_t = outs.tile([P, NQ], F32, tag="ot")
                    nc.vector.tensor_add(out=out_t, in0=tmp, in1=xmid[:, m, :])
                    nc.sync.dma_start(out=yt_r[:, m, :], in_=out_t)

    nc.compile()
    return nc


def _prep_weight(w, mtiles):
    """[Cin, Cout] -> [mtiles, 128, Cin/128 * 128] tile-contiguous bf16."""
    cin, cout = w.shape
    a = cin // P
    r = w.reshape(a, P, mtiles, P).transpose(2, 1, 0, 3).reshape(mtiles, P, a * P)
    return np.ascontiguousarray(r).astype(ml_dtypes.bfloat16)


def _col_table(*vecs):
    """each vec [C] -> columns of [128, 8] chunk-major; concat along axis 1."""
    cols = [v.reshape(-1, P).T for v in vecs]  # [128, 8] each
    return np.ascontiguousarray(np.concatenate(cols, axis=1)).astype(np.float32)


def prepare_in_maps(x, ln1_g, ln1_b, wq, bq, wk, bk, wv, bv, wo, bo,
                    ln2_g, ln2_b, w1, b1, w2, b2):
    x = np.asarray(x, np.float32)
    wq_p = _prep_weight(np.asarray(wq, np.float32), CH)
    wk_p = _prep_weight(np.asarray(wk, np.float32), CH)
    wv_p = _prep_weight(np.asarray(wv, np.float32), CH)
    wo_p = _prep_weight(np.asarray(wo, np.float32), CH)
    w1_p = _prep_weight(np.asarray(w1, np.float32), FH)
    w2_p = _prep_weight(np.asarray(w2, np.float32), CH)
    bias_tab = _col_table(np.asarray(bq, np.float32), np.asarray(bk, np.float32),
                          np.asarray(ln2_b, np.float32), np.asarray(bo, np.float32),
                          np.asarray(b2, np.float32), np.asarray(ln1_g, np.float32),
                          np.asarray(ln1_b, np.float32), np.asarray(ln2_g, np.float32))
    b1_tab = np.ascontiguousarray(np.asarray(b1, np.float32).reshape(FH, P).T)
    bv_full = np.asarray(bv, np.float32).reshape(1, C)

    in_maps = []
    for i in range(NCORES):
        n0 = (i + 1) * QC
        n1 = (NCORES - i) * QC
        b0q = x[0, n0 - QC:n0]
        b1q = x[1, n1 - QC:n1]
        b0r = x[0, 0:n0 - QC]
        b1r = x[1, 0:n1 - QC]
        xk = np.concatenate([b0q, b1q, b0r, b1r], 0)       # [2304, 1024]
        xt_i = np.ascontiguousarray(xk.T)                  # [1024, 2304]
        kb = np.zeros((NK, 2), np.float32)
        kb[NQ:NQ + (n0 - QC), 1] = NEG        # b0-rest: masked for b1 queries
        kb[NQ + (n0 - QC):, 0] = NEG          # b1-rest: masked for b0 queries
        kb_i = np.ascontiguousarray(kb.reshape(NK // P, P, 2).transpose(1, 0, 2))
        in_maps.append({
            "xt": xt_i, "wq": wq_p, "wk": wk_p, "wv": wv_p, "wo": wo_p,
            "w1": w1_p, "w2": w2_p, "bias": bias_tab, "b1": b1_tab,
            "kbias": kb_i, "bvfull": bv_full,
        })
    return in_maps


def assemble_output(per_core_yt):
    out = np.empty((B, T, C), np.float32)
    for i in range(NCORES):
        yt_i = np.asarray(per_core_yt[i])                  # [1024, 512]
        n0 = (i + 1) * QC
        n1 = (NCORES - i) * QC
        out[0, n0 - QC:n0] = yt_i[:, 0:QC].T
        out[1, n1 - QC:n1] = yt_i[:, QC:NQ].T
    return out


def kernel(**inputs):
    if "nc" not in _NC_CACHE:
        _NC_CACHE["nc"] = _build_nc()
    nc = _NC_CACHE["nc"]
    in_maps = prepare_in_maps(**inputs)
    res = run_bass_kernel_spmd(nc, in_maps, core_ids=list(range(NCORES)))
    return assemble_output([res.results[i]["yt"] for i in range(NCORES)])
